# revision 8
# baseline (speedup 1.0000x reference)
"""Trainium2 Bass kernel for a 2-layer ResGatedGraphConv encoder.

Strategy (edge-parallel over 8 NeuronCores):
  - Nodes are permuted by degree rank and dealt round-robin to the 8 cores, so
    each core owns NPC nodes arranged in TPC tiles of 128 dst nodes whose
    degrees are nearly uniform within a tile.
  - Each edge lives on the core/tile/partition of its dst node.  The [q|v]
    node table is split into 4 row windows of 2 cores each (25088 rows, so
    int16 dma_gather indices reach every row); per edge the window is fixed
    by its src node.  Tiles are packed into groups; per (group, window) ONE
    batched dma_gather fetches all edge rows token-major, with per-
    (tile, window) degree padded to the group max (padding gathers an
    all-zero table row, so sums are unaffected).
  - Per layer the table [NT, 128] (bf16) is built on device with PE matmuls.
    k is only needed per dst node: computed per tile [128, 64] and broadcast
    along the degree axis.  Messages: per-window group adds (k broadcast),
    one sigmoid + multiply per group; fold trees reduce the degree axis;
    per-tile PE transpose + fused linear.
  - h1 is exchanged between cores with an AllGather (bf16) so layer 2 can
    build its node table from the full hidden state.  Unused node slots of
    h1 are explicitly zeroed so padding rows stay zero in layer 2.
  - Bias algebra is folded on the host: (agg + x@Ws + b) @ Wl + bl
    = agg@Wl + x@(Ws@Wl) + (b@Wl + bl).

kernel(**inputs) takes the full (unsharded) inputs and returns the full
output; all sharding happens inside.
"""

import os
import sys
import numpy as np

for _p in ("/opt/trn_rl_repo", "/opt/pypackages"):
    if _p not in sys.path:
        sys.path.append(_p)

N = 100000
E = 1600000
H = 64
NCORES = 8
NCHUNK = 4       # table row windows (int16 index reach)
GCOLS = 128      # max padded columns per gather group
GTILES = 8       # max tiles per group (ksb PSUM bank: T*64 <= 512)


class Cfg:
    def __init__(self, n, tpc):
        self.n = n
        self.tpc = tpc                      # dst tiles per core
        self.npc = tpc * 128                # nodes per core
        self.npad = NCORES * self.npc       # padded node count
        self.nt = self.npad                 # table rows
        self.chw = self.npad // NCHUNK      # rows per window
        assert self.npad >= n
        assert self.npad % 512 == 0
        assert self.chw <= 32768
        # per-core used slots (nodes actually assigned); rest are zero rows
        assert n % NCORES == 0
        self.used = n // NCORES
        assert self.used < self.npc


FULL_CFG = Cfg(N, 98)


def make_groups(dh_tc):
    """Pack consecutive tiles into groups; per window pad to the group max.

    dh_tc: [tpc, NCHUNK] per-(tile, window) max degree (>=1).
    Returns list of group dicts.
    """
    tpc = len(dh_tc)
    groups = []
    t0 = 0
    while t0 < tpc:
        T = 1
        dh_g = list(dh_tc[t0])
        while t0 + T < tpc and T < GTILES:
            cand = [max(a, b) for a, b in zip(dh_g, dh_tc[t0 + T])]
            if (T + 1) * sum(cand) > GCOLS:
                break
            dh_g = cand
            T += 1
        groups.append(dict(t0=t0, T=T, dh=dh_g, cols=T * sum(dh_g)))
        t0 += T
    return groups


def host_prep(x, edge_index, cfg):
    """Permute nodes / build per-core gather schedules on the host."""
    n = cfg.n
    src = np.asarray(edge_index[0]).astype(np.int64)
    dst = np.asarray(edge_index[1]).astype(np.int64)
    deg = np.bincount(dst, minlength=n)

    # degree-rank round-robin: rank r -> core r%8, slot r//8
    rank_order = np.argsort(deg, kind="stable")  # node ids in degree order
    r = np.arange(n)
    node_core = np.empty(n, np.int64)
    node_slot = np.empty(n, np.int64)
    node_core[rank_order] = r % NCORES
    node_slot[rank_order] = r // NCORES
    tau = node_core * cfg.npc + node_slot      # table id of each node

    e_core = node_core[dst]
    e_slot = node_slot[dst]
    e_tile = e_slot // 128
    e_part = e_slot % 128
    tau_src = tau[src]
    e_chunk = tau_src // cfg.chw

    # per-edge position within its (dst, window) list
    okey = (e_core * cfg.npc + e_slot) * NCHUNK + e_chunk
    order = np.argsort(okey, kind="stable")
    ok_sorted = okey[order]
    uniq, first = np.unique(ok_sorted, return_index=True)
    k_within = np.arange(len(dst)) - first[np.searchsorted(uniq, ok_sorted)]
    k_e = np.empty(len(dst), np.int64)
    k_e[order] = k_within

    # per-(tile, window) max degree (shared across cores: identical program)
    cnt = np.zeros((NCORES, cfg.tpc, 128, NCHUNK), np.int32)
    np.add.at(cnt, (e_core, e_tile, e_part, e_chunk), 1)
    dh_tc = np.maximum(cnt.max(axis=(0, 2)), 1)     # [tpc, NCHUNK]

    groups = make_groups(dh_tc.tolist())

    # zero table row (relative to each window): first unused slot of the
    # window's first core
    zr_rel = cfg.used            # slot `used` of core 2c -> rel = used
    assert zr_rel < cfg.chw

    # slot column assignment within the group-padded layout
    # col index inside group g: seg_off[c] + ti*dh_g[c] + k_e
    tile_group = np.zeros(cfg.tpc, np.int64)
    tile_ti = np.zeros(cfg.tpc, np.int64)
    for gi, g in enumerate(groups):
        for i in range(g["T"]):
            tile_group[g["t0"] + i] = gi
            tile_ti[g["t0"] + i] = i

    # build per-core int16 index blocks, one per (group, window)
    total_cols = sum(g["cols"] for g in groups)
    # slot grid per core: [128 part, total_cols] holding relative table row
    grid = np.full((NCORES, 128, total_cols), -1, np.int32)
    gcol0 = {}
    c0 = 0
    for gi, g in enumerate(groups):
        gcol0[gi] = c0
        c0 += g["cols"]

    e_g = tile_group[e_tile]
    # segment offset of (group, window)
    seg_off = np.zeros((len(groups), NCHUNK), np.int64)
    for gi, g in enumerate(groups):
        off = 0
        for c in range(NCHUNK):
            seg_off[gi, c] = off
            off += g["T"] * g["dh"][c]
    dh_arr = np.zeros((len(groups), NCHUNK), np.int64)
    for gi, g in enumerate(groups):
        dh_arr[gi] = g["dh"]

    col = (np.array([gcol0[gi] for gi in e_g])
           + seg_off[e_g, e_chunk]
           + tile_ti[e_tile] * dh_arr[e_g, e_chunk]
           + k_e)
    grid[e_core, e_part, col] = (tau_src - e_chunk * cfg.chw).astype(np.int32)

    # wrapped int16 index tensor per core: per (group, window) block of
    # ceil(nidx/16) columns, nidx = 128 * seg_cols, order i=(j*128+p)
    blocks = []
    meta = []   # (group, window, seg_cols, idxcol0, ncols16)
    idxcol = 0
    for gi, g in enumerate(groups):
        for c in range(NCHUNK):
            seg_cols = g["T"] * g["dh"][c]
            csta = gcol0[gi] + seg_off[gi, c]
            sub = grid[:, :, csta:csta + seg_cols]     # [NCORES, 128, cols]
            # flat i = j*128 + p  -> [cols, 128] -> ravel
            flat = np.transpose(sub, (0, 2, 1)).reshape(NCORES, -1)
            flat = np.where(flat < 0, zr_rel, flat)
            nidx = flat.shape[1]
            ncols16 = (nidx + 15) // 16
            pad = ncols16 * 16 - nidx
            if pad:
                flat = np.concatenate(
                    [flat, np.full((NCORES, pad), zr_rel, np.int32)], axis=1)
            wrapped = flat.reshape(NCORES, ncols16, 16).transpose(0, 2, 1)
            wrapped = np.tile(wrapped, (1, 8, 1))      # replicate to 128 part
            blocks.append(wrapped.astype(np.int16))
            meta.append(dict(g=gi, c=c, seg_cols=seg_cols,
                             idxcol0=idxcol, ncols16=ncols16))
            idxcol += ncols16
    qidx = np.concatenate(blocks, axis=2)              # [NCORES, 128, idxcol]

    # permuted feature table input, feature-major, zero padded, bf16
    bf16 = _bf16_dtype()
    xT_full = np.zeros((H, cfg.nt), np.float32)
    xT_full[:, tau] = np.asarray(x, np.float32).T
    xT_full = xT_full.astype(bf16)

    return dict(
        qidx=qidx,
        groups=groups,
        meta=meta,
        nidxcols=idxcol,
        tau=tau,
        xT_full=xT_full,
    )


def _bf16_dtype():
    import concourse.mybir as mybir
    return mybir.dt.np(mybir.dt.bfloat16)


def build_program(cfg, groups, meta, nidxcols):
    import concourse.bass as bass
    import concourse.bacc as bacc
    import concourse.mybir as mybir
    import concourse.tile as tile
    from concourse.masks import make_identity

    f32 = mybir.dt.float32
    bf16 = mybir.dt.bfloat16
    i16 = mybir.dt.int16
    tpc, npc, nt = cfg.tpc, cfg.npc, cfg.nt
    npad = cfg.npad

    # meta lookup: per group, list of window entries
    gmeta = [[] for _ in groups]
    for m in meta:
        gmeta[m["g"]].append(m)

    nc = bacc.Bacc("TRN2", target_bir_lowering=False, debug=False,
                   num_devices=NCORES)

    # ---- I/O ----
    xT_full = nc.dram_tensor("xT_full", [H, nt], bf16, kind="ExternalInput")
    xT_own = nc.dram_tensor("xT_own", [H, npc], bf16, kind="ExternalInput")
    qidx = nc.dram_tensor("qidx", [128, nidxcols], i16, kind="ExternalInput")
    wnames = {}
    for l in (1, 2):
        for w in ("Wqv", "Wk", "Wsl", "Wl"):
            shape = [H, 128] if w == "Wqv" else [H, H]
            wnames[f"{w}{l}"] = nc.dram_tensor(f"{w}{l}", shape, bf16,
                                               kind="ExternalInput")
        wnames[f"blp{l}"] = nc.dram_tensor(f"blp{l}", [H, 1], f32,
                                           kind="ExternalInput")
    out_shard = nc.dram_tensor("out_shard", [H, npc], f32,
                               kind="ExternalOutput")

    # ---- internal DRAM ----
    qv_t = [nc.dram_tensor(f"qv{l}_t", [nt, 128], bf16) for l in (1, 2)]
    h1_shard = nc.dram_tensor("h1_shard", [H, npc], bf16)
    h1_gath = nc.dram_tensor("h1_gath", [NCORES * H, npc], bf16,
                             addr_space="Shared")

    with tile.TileContext(nc) as tc:
        cp = tc.alloc_tile_pool(name="const", bufs=1)

        identity = cp.tile([128, 128], f32)
        make_identity(nc, identity[:])

        wt = {}
        for l in (1, 2):
            for w in ("Wqv", "Wk", "Wsl", "Wl"):
                shape = [H, 128] if w == "Wqv" else [H, H]
                wt[f"{w}{l}"] = cp.tile(shape, bf16, name=f"{w}{l}",
                                        tag=f"{w}{l}")
                nc.sync.dma_start(out=wt[f"{w}{l}"][:],
                                  in_=wnames[f"{w}{l}"].ap()[:])
            wt[f"blp{l}"] = cp.tile([H, 1], f32, name=f"blp{l}", tag=f"blp{l}")
            nc.sync.dma_start(out=wt[f"blp{l}"][:],
                              in_=wnames[f"blp{l}"].ap()[:])

        # ---------------- phase A: build [q|v] table ----------------
        CH = 4  # table tiles per chunk (one PSUM bank: 4*128 = 512 fp32)

        def phase_a(layer):
            table = qv_t[layer - 1]
            wqv = wt[f"Wqv{layer}"]
            ntiles = npad // 128
            with tc.tile_pool(name=f"pa{layer}", bufs=3) as pa, \
                 tc.tile_pool(name=f"pap{layer}", bufs=2, space="PSUM") as pap:
                for j0 in range(0, ntiles, CH):
                    src_t = pa.tile([H, CH * 128], bf16, tag="src")
                    if layer == 1:
                        nc.sync.dma_start(
                            out=src_t[:],
                            in_=xT_full.ap()[:, j0 * 128:(j0 + CH) * 128])
                    else:
                        # source h1_gath with per-core reslicing; a chunk may
                        # cross a core boundary
                        seg0 = 0
                        while seg0 < CH * 128:
                            g = j0 * 128 + seg0
                            c = g // npc
                            n0 = g % npc
                            seglen = min(CH * 128 - seg0, npc - n0)
                            nc.sync.dma_start(
                                out=src_t[:, seg0:seg0 + seglen],
                                in_=h1_gath.ap()[c * H:(c + 1) * H,
                                                 n0:n0 + seglen])
                            seg0 += seglen
                    ps = pap.tile([128, CH * 128], f32, tag="ps", space="PSUM")
                    for i in range(CH):
                        nc.tensor.matmul(
                            ps[:, i * 128:(i + 1) * 128],
                            lhsT=src_t[:, i * 128:(i + 1) * 128],
                            rhs=wqv[:], start=True, stop=True)
                    st = pa.tile([128, CH * 128], bf16, tag="st")
                    nc.scalar.activation(st[:], ps[:],
                                         mybir.ActivationFunctionType.Copy)
                    out_ap = table.ap()[j0 * 128:(j0 + CH) * 128, :]
                    out_ap = out_ap.rearrange("(c p) e -> p c e", p=128)
                    in_ap = st[:].rearrange("p (c e) -> p c e", e=128)
                    nc.sync.dma_start(out=out_ap, in_=in_ap)

        # ---------------- phase B: gated conv + fused linear ----------------
        def conv_layer(layer):
            table = qv_t[layer - 1]
            hsrc_dram = xT_own if layer == 1 else h1_shard
            wk, wsl, wl = wt[f"Wk{layer}"], wt[f"Wsl{layer}"], wt[f"Wl{layer}"]
            blp = wt[f"blp{layer}"]
            odram = h1_shard if layer == 1 else out_shard
            odt = bf16 if layer == 1 else f32
            with tc.tile_pool(name=f"pb{layer}", bufs=2) as pb, \
                 tc.tile_pool(name=f"pbp{layer}", bufs=2, space="PSUM") as pbp:
                for gi, g in enumerate(groups):
                    t0, T, dh = g["t0"], g["T"], g["dh"]
                    C = g["cols"]
                    # per-group loads
                    hot = pb.tile([H, GTILES * 128], bf16, tag="hot")
                    nc.sync.dma_start(
                        out=hot[:, 0:T * 128],
                        in_=hsrc_dram.ap()[:, t0 * 128:(t0 + T) * 128])
                    mm = gmeta[gi]
                    i0 = mm[0]["idxcol0"]
                    iN = mm[-1]["idxcol0"] + mm[-1]["ncols16"]
                    qix = pb.tile([128, (GCOLS * 128) // 16 + 16], i16,
                                  tag="qix")
                    nc.sync.dma_start(out=qix[:, 0:iN - i0],
                                      in_=qidx.ap()[:, i0:iN])
                    qvg_f = pb.tile([128, GCOLS * 128], bf16, tag="qvg")
                    CAPC = 100   # <=12800 idxs per call (descriptor carveout)
                    off = 0
                    for m in mm:
                        segc = m["seg_cols"]
                        for a0 in range(0, segc, CAPC):
                            sc = min(CAPC, segc - a0)
                            nidx = 128 * sc
                            o = off + a0
                            nc.gpsimd.dma_gather(
                                out_ap=qvg_f[:, o * 128:(o + sc) * 128]
                                .rearrange("p (c e) -> p c e", e=128),
                                in_ap=table.ap()[m["c"] * cfg.chw:
                                                 (m["c"] + 1) * cfg.chw, :],
                                idxs_ap=qix[:, m["idxcol0"] - i0 + a0 * 8:
                                            m["idxcol0"] - i0 + (a0 + sc) * 8],
                                num_idxs=nidx,
                                num_idxs_reg=nidx,
                                elem_size=128,
                                single_packet=False,
                            )
                        off += segc

                    # k for all tiles of the group: PSUM bank [128, T*64]
                    kps = pbp.tile([128, GTILES * H], f32, tag="kps",
                                   space="PSUM")
                    for i in range(T):
                        nc.tensor.matmul(
                            kps[:, i * H:(i + 1) * H],
                            lhsT=hot[:, i * 128:(i + 1) * 128],
                            rhs=wk[:], start=True, stop=True)
                    ksb = pb.tile([128, GTILES * H], bf16, tag="ksb")
                    nc.scalar.activation(ksb[:, 0:T * H], kps[:, 0:T * H],
                                         mybir.ActivationFunctionType.Copy)

                    # sigarg = q + k[dst]: one op per window segment
                    sigarg = pb.tile([128, GCOLS * H], bf16, tag="sigarg")
                    kb0 = ksb[:, 0:T * H].rearrange(
                        "p (t o h) -> p t o h", o=1, h=H)
                    off = 0
                    for c in range(len(dh)):
                        dhc = dh[c]
                        qv = qvg_f[:, off * 128:(off + T * dhc) * 128] \
                            .rearrange("p (t k e) -> p t k e", k=dhc, e=128)
                        sa = sigarg[:, off * H:(off + T * dhc) * H].rearrange(
                            "p (t k h) -> p t k h", k=dhc, h=H)
                        kb = bass.AP(kb0.tensor, kb0.offset,
                                     [kb0.ap[0], kb0.ap[1], [0, dhc],
                                      kb0.ap[3]])
                        nc.vector.tensor_tensor(
                            out=sa, in0=qv[:, :, :, 0:H], in1=kb,
                            op=mybir.AluOpType.add)
                        off += T * dhc
                    # sig = sigmoid(sigarg), one op for the whole group
                    sig = pb.tile([128, GCOLS * H], bf16, tag="sig")
                    nc.scalar.activation(
                        sig[:, 0:C * H], sigarg[:, 0:C * H],
                        mybir.ActivationFunctionType.Sigmoid)
                    # msg = sig * v, one op for the whole group
                    # (reuse sigarg as msg storage)
                    msg = sigarg
                    nc.vector.tensor_tensor(
                        out=msg[:, 0:C * H].rearrange(
                            "p (a h) -> p a h", h=H),
                        in0=sig[:, 0:C * H].rearrange(
                            "p (a h) -> p a h", h=H),
                        in1=qvg_f[:, 0:C * 128].rearrange(
                            "p (a e) -> p a e", e=128)[:, :, H:128],
                        op=mybir.AluOpType.mult)

                    # fold tree per window -> one head column per window,
                    # then combine heads into agg (f32)
                    agg = pb.tile([128, GTILES * H], f32, tag="agg")
                    av = agg[:, 0:T * H].rearrange("p (t h) -> p t h", h=H)
                    heads = []
                    off = 0
                    for c in range(len(dh)):
                        dhc = dh[c]
                        mv = msg[:, off * H:(off + T * dhc) * H].rearrange(
                            "p (t k h) -> p t k h", k=dhc, h=H)
                        cur = dhc
                        while cur > 1:
                            k2 = cur // 2
                            nc.vector.tensor_tensor(
                                out=mv[:, :, 0:k2, :],
                                in0=mv[:, :, 0:k2, :],
                                in1=mv[:, :, cur - k2:cur, :],
                                op=mybir.AluOpType.add)
                            cur -= k2
                        heads.append(mv[:, :, 0, :])
                        off += T * dhc
                    # combine 4 heads: (h0+h1) + (h2+h3)
                    h01 = pb.tile([128, GTILES * H], bf16, tag="h01")
                    hv = h01[:, 0:T * H].rearrange("p (t h) -> p t h", h=H)
                    nc.vector.tensor_tensor(out=hv, in0=heads[0],
                                            in1=heads[1],
                                            op=mybir.AluOpType.add)
                    nc.vector.tensor_tensor(out=heads[2], in0=heads[2],
                                            in1=heads[3],
                                            op=mybir.AluOpType.add)
                    nc.vector.tensor_tensor(out=av, in0=hv, in1=heads[2],
                                            op=mybir.AluOpType.add)

                    # transpose agg tiles to feature-major, 4 tiles per bank
                    ob = pb.tile([H, GTILES * 128], odt, tag="ob")
                    for b0 in range(0, T, 4):
                        nb = min(4, T - b0)
                        tps = pbp.tile([H, 4 * 128], f32, tag="tps",
                                       space="PSUM")
                        for i in range(nb):
                            nc.tensor.transpose(
                                out=tps[:, i * 128:(i + 1) * 128],
                                in_=agg[:, (b0 + i) * H:(b0 + i + 1) * H],
                                identity=identity[:])
                        aggT = pb.tile([H, 4 * 128], bf16, tag="aggT")
                        nc.scalar.activation(
                            aggT[:, 0:nb * 128], tps[:, 0:nb * 128],
                            mybir.ActivationFunctionType.Copy)
                        # fused linear: agg@Wl + x@(Ws@Wl)
                        lps = pbp.tile([H, 4 * 128], f32, tag="lps",
                                       space="PSUM")
                        nc.tensor.matmul(lps[:, 0:nb * 128], lhsT=wl[:],
                                         rhs=aggT[:, 0:nb * 128],
                                         start=True, stop=False)
                        nc.tensor.matmul(
                            lps[:, 0:nb * 128], lhsT=wsl[:],
                            rhs=hot[:, b0 * 128:(b0 + nb) * 128],
                            start=False, stop=True)
                        nc.scalar.activation(
                            ob[:, b0 * 128:(b0 + nb) * 128],
                            lps[:, 0:nb * 128],
                            mybir.ActivationFunctionType.Relu,
                            bias=blp[:])
                    nc.sync.dma_start(
                        out=odram.ap()[:, t0 * 128:(t0 + T) * 128],
                        in_=ob[:, 0:T * 128])

        phase_a(1)
        tc.strict_bb_all_engine_barrier()
        conv_layer(1)

        # zero unused h1 columns so layer-2 table zero rows stay zero
        zt = cp.tile([H, 128], bf16)
        nc.vector.memset(zt[:], 0.0)
        nc.sync.dma_start(out=h1_shard.ap()[:, cfg.used:npc],
                          in_=zt[:, 0:npc - cfg.used])

        # exchange h1 across cores
        nc.gpsimd.collective_compute(
            "AllGather",
            mybir.AluOpType.bypass,
            replica_groups=[list(range(NCORES))],
            ins=[h1_shard.ap()[:, :]],
            outs=[h1_gath.ap()[:, :]],
        )

        phase_a(2)
        tc.strict_bb_all_engine_barrier()
        conv_layer(2)

        cp.release()

    nc.compile()
    return nc


def _pack_inputs(prep, inputs, cfg):
    """Build the 8 per-core input maps."""
    bf16 = _bf16_dtype()
    xT_full = prep["xT_full"]
    base = {"xT_full": xT_full}
    for l, (wq, wv, wk, ws, b, wl, bl) in {
        1: ("Wq1", "Wv1", "Wk1", "Ws1", "b1", "Wl1", "bl1"),
        2: ("Wq2", "Wv2", "Wk2", "Ws2", "b2", "Wl2", "bl2"),
    }.items():
        Wq = np.asarray(inputs[wq], np.float32)
        Wv = np.asarray(inputs[wv], np.float32)
        Wk = np.asarray(inputs[wk], np.float32)
        Ws = np.asarray(inputs[ws], np.float32)
        Wl = np.asarray(inputs[wl], np.float32)
        b = np.asarray(inputs[b], np.float32)
        bl = np.asarray(inputs[bl], np.float32)
        base[f"Wqv{l}"] = np.ascontiguousarray(
            np.concatenate([Wq, Wv], axis=1)).astype(bf16)
        base[f"Wk{l}"] = np.ascontiguousarray(Wk).astype(bf16)
        base[f"Wsl{l}"] = np.ascontiguousarray(Ws @ Wl).astype(bf16)
        base[f"Wl{l}"] = np.ascontiguousarray(Wl).astype(bf16)
        base[f"blp{l}"] = np.ascontiguousarray(
            (b @ Wl + bl).reshape(H, 1).astype(np.float32))

    in_maps = []
    for c in range(NCORES):
        m = dict(base)
        m["xT_own"] = np.ascontiguousarray(
            xT_full[:, c * cfg.npc:(c + 1) * cfg.npc])
        m["qidx"] = np.ascontiguousarray(prep["qidx"][c])
        in_maps.append(m)
    return in_maps


def run(inputs, cfg=FULL_CFG, sim=False, trace=False):
    from concourse import bass_utils

    x = np.asarray(inputs["x"], np.float32)
    prep = host_prep(x, inputs["edge_index"], cfg)
    nc = build_program(cfg, prep["groups"], prep["meta"], prep["nidxcols"])
    in_maps = _pack_inputs(prep, inputs, cfg)

    if sim:
        from concourse.bass_interp import MultiCoreSim
        ms = MultiCoreSim(nc, num_cores=NCORES, trace=False)
        for c in range(NCORES):
            for name, arr in in_maps[c].items():
                ms.cores[c].tensor(name)[:] = arr
        ms.simulate(check_with_hw=False)
        shards = [np.array(ms.cores[c].tensor("out_shard")) for c in
                  range(NCORES)]
        res = None
    else:
        if trace:
            try:
                sys.path.insert(0, "/root/problem")
                import ntff_hook  # noqa: F401
            except Exception:
                trace = False
        res = bass_utils.run_bass_kernel_spmd(
            nc, in_maps, core_ids=list(range(NCORES)), trace=trace)
        shards = [res.results[c]["out_shard"] for c in range(NCORES)]

    full_T = np.concatenate(shards, axis=1)   # [H, npad] in permuted order
    out = np.ascontiguousarray(full_T[:, prep["tau"]].T.astype(np.float32))
    return out, res


def kernel(**inputs):
    out, _ = run(inputs, FULL_CFG, sim=False, trace=False)
    return out.astype(np.float32)
